# revision 12
# baseline (speedup 1.0000x reference)
"""Bass/Trainium2 kernel for nn_GraphSToV (gnn_message_passing).

Computes, for full inputs:
  scalar_features [B=8, N=128, F=128] f32
  distances       [B=8, N=128, N=128, C=3] f32
  W               [2F=256, K=128] f32
  b               [1, K=128] f32
Output:
  vector_features [B, N, N, C, K] f32
  = (h_i @ W1 + h_j @ W2 + b)[b,i,j,k] * distances[b,i,j,c]

Sharding: data-parallel over batch B across 8 NeuronCores (1 batch each).

Per-core dataflow (partition dim = i everywhere):
  hT   = transpose(h)                    (PE, via identity matmul)
  s2b  = h @ W2 + bias                   (PE, bias via rank-1 ones x b)
  for j:  pair_j = h @ W1 + ones x s2b[j]   (PE, PSUM accumulate)
          pair_sb = copy(pair_psum)         (ACT)
          out[:, j, c, :] = pair_sb * dist[:, j, c]  (DVE tensor_scalar, x3)
  out written in groups of JG j's => contiguous [128, JG*384] DMA per group
  (out[b, i, :, :, :] is fully contiguous in DRAM for fixed i).
"""

import numpy as np
from contextlib import ExitStack

import concourse.bass as bass
import concourse.bacc as bacc
import concourse.mybir as mybir
import concourse.tile as tile
from concourse.bass_utils import run_bass_kernel_spmd
from concourse.masks import make_identity

B, N, F, C, K = 8, 128, 128, 3, 128
JG = 8            # j's per output group (one DMA per group)
NG = N // JG      # number of groups
F32 = mybir.dt.float32

_CACHE = {}


def _build_nc(reps=1):
    # Bacc (not raw Bass): its finalize() runs move_matmul_waits_to_ldweights
    # + generate_event_semaphores, which legalize multi-wait instructions for
    # the TRN2 one-wait-per-instruction ISA constraint.
    nc = bacc.Bacc()
    h_d = nc.declare_dram_parameter("h", [N, F], F32, isOutput=False)
    dist_d = nc.declare_dram_parameter("dist", [N, N * C], F32, isOutput=False)
    W_d = nc.declare_dram_parameter("W", [2 * F, K], F32, isOutput=False)
    b_d = nc.declare_dram_parameter("b", [1, K], F32, isOutput=False)
    out_d = nc.declare_dram_parameter("out", [N, N * C * K], F32, isOutput=True)

    with tile.TileContext(nc) as tc, ExitStack() as ctx:
        const = ctx.enter_context(tc.tile_pool(name="const", bufs=1))
        psum_setup = ctx.enter_context(tc.tile_pool(name="psum_setup", bufs=1, space="PSUM"))
        psum_pair = ctx.enter_context(tc.tile_pool(name="psum_pair", bufs=3, space="PSUM"))
        sb_pair = ctx.enter_context(tc.tile_pool(name="sb_pair", bufs=3))
        sb_out = ctx.enter_context(tc.tile_pool(name="sb_out", bufs=3))

        h_t = const.tile([N, F], F32)
        nc.sync.dma_start(h_t[:], h_d[:])
        dist_t = const.tile([N, N * C], F32)
        nc.sync.dma_start(dist_t[:], dist_d[:])
        W1_t = const.tile([F, K], F32)
        nc.sync.dma_start(W1_t[:], W_d[0:F, :])
        W2_t = const.tile([F, K], F32)
        nc.sync.dma_start(W2_t[:], W_d[F:2 * F, :])
        b_t = const.tile([1, K], F32)
        nc.sync.dma_start(b_t[:], b_d[:])

        ones_t = const.tile([1, N], F32)
        nc.vector.memset(ones_t[:], 1.0)
        ident = const.tile([N, N], F32)
        make_identity(nc, ident[:])

        # hT = h.T  (PE transpose)
        hT_ps = psum_setup.tile([F, N], F32, tag="hT_ps")
        nc.tensor.transpose(hT_ps[:], h_t[:], ident[:])
        hT_t = const.tile([F, N], F32)
        nc.scalar.copy(hT_t[:], hT_ps[:])

        # s2b = h @ W2 + bias (broadcast along partitions via ones x b)
        s2b_ps = psum_setup.tile([N, K], F32, tag="s2b_ps")
        nc.tensor.matmul(s2b_ps[:], hT_t[:], W2_t[:], start=True, stop=False)
        nc.tensor.matmul(s2b_ps[:], ones_t[:], b_t[:], start=False, stop=True)
        s2b_t = const.tile([N, K], F32)
        nc.scalar.copy(s2b_t[:], s2b_ps[:])
        # Flatten s2b rows onto partition 0 so row j is a free-dim slice
        # (compute-engine APs may only base at partition 0/32/64).
        s2b_flat = const.tile([1, N * K], F32)
        nc.sync.dma_start(s2b_flat[:], s2b_t[:])

        for g in range(NG * reps):
            g = g % NG
            pair_sb = sb_pair.tile([N, JG * K], F32, tag="pair")
            for half in range(JG // 4):
                pp = psum_pair.tile([N, 4 * K], F32, tag="pp")
                j0 = g * JG + half * 4
                for q in range(4):
                    sl = pp[:, q * K:(q + 1) * K]
                    nc.tensor.matmul(sl, hT_t[:], W1_t[:], start=True, stop=False)
                    nc.tensor.matmul(sl, ones_t[:],
                                     s2b_flat[0:1, (j0 + q) * K:(j0 + q + 1) * K],
                                     start=False, stop=True)
                nc.scalar.copy(pair_sb[:, half * 4 * K:(half + 1) * 4 * K], pp[:])

            out_sb = sb_out.tile([N, JG * C * K], F32, tag="out")
            for dj in range(JG):
                j = g * JG + dj
                for c in range(C):
                    nc.vector.tensor_scalar_mul(
                        out_sb[:, (dj * C + c) * K:(dj * C + c + 1) * K],
                        pair_sb[:, dj * K:(dj + 1) * K],
                        dist_t[:, C * j + c:C * j + c + 1],
                    )
            nc.sync.dma_start(
                out_d[:, g * JG * C * K:(g + 1) * JG * C * K], out_sb[:])
    nc.finalize()
    return nc


def _run(scalar_features, distances, W, b, trace=False, reps=1):
    if ("nc", reps) not in _CACHE:
        _CACHE[("nc", reps)] = _build_nc(reps)
    nc = _CACHE[("nc", reps)]
    in_maps = []
    for i in range(B):
        in_maps.append({
            "h": np.ascontiguousarray(scalar_features[i], dtype=np.float32),
            "dist": np.ascontiguousarray(
                distances[i].reshape(N, N * C), dtype=np.float32),
            "W": np.ascontiguousarray(W, dtype=np.float32),
            "b": np.ascontiguousarray(np.asarray(b).reshape(1, K), dtype=np.float32),
        })
    r = run_bass_kernel_spmd(nc, in_maps, list(range(B)), trace=trace)
    out = np.stack([r.results[i]["out"].reshape(N, N, C, K) for i in range(B)])
    return out, r


def kernel(scalar_features, distances, W, b):
    out, _ = _run(scalar_features, distances, W, b, trace=False)
    return out


# revision 15
# speedup vs baseline: 5.6456x; 5.6456x over previous
"""Bass/Trainium2 kernel for nn_GraphSToV (gnn_message_passing).

Computes, for full inputs:
  scalar_features [B=8, N=128, F=128] f32
  distances       [B=8, N=128, N=128, C=3] f32
  W               [2F=256, K=128] f32
  b               [1, K=128] f32
Output:
  vector_features [B, N, N, C, K] f32
  = (h_i @ W1 + h_j @ W2 + b)[b,i,j,k] * distances[b,i,j,c]

Sharding: data-parallel over batch B across 8 NeuronCores (1 batch each).

Per-core dataflow (partition dim = i everywhere):
  hT   = transpose(h)                    (PE, via identity matmul)
  s2b  = h @ W2 + bias                   (PE, bias via rank-1 ones x b)
  for j:  pair_j = h @ W1 + ones x s2b[j]   (PE, PSUM accumulate)
          pair_sb = copy(pair_psum)         (ACT)
          out[:, j, c, :] = pair_sb * dist[:, j, c]  (DVE tensor_scalar, x3)
  out written in groups of JG j's => contiguous [128, JG*384] DMA per group
  (out[b, i, :, :, :] is fully contiguous in DRAM for fixed i).
"""

import numpy as np
from contextlib import ExitStack

import concourse.bass as bass
import concourse.bacc as bacc
import concourse.mybir as mybir
import concourse.tile as tile
from concourse.bass_utils import run_bass_kernel_spmd
from concourse.masks import make_identity

B, N, F, C, K = 8, 128, 128, 3, 128
JG = 8            # j's per output group (one DMA per group)
NG = N // JG      # number of groups
F32 = mybir.dt.float32

_CACHE = {}


def _build_nc(reps=1):
    # Bacc (not raw Bass): its finalize() runs move_matmul_waits_to_ldweights
    # + generate_event_semaphores, which legalize multi-wait instructions for
    # the TRN2 one-wait-per-instruction ISA constraint.
    nc = bacc.Bacc()
    F32R = mybir.dt.float32r
    h_d = nc.declare_dram_parameter("h", [N, F], F32, isOutput=False)
    dist_d = nc.declare_dram_parameter("dist", [N, N * C], F32, isOutput=False)
    # W/b feed the PE only: declare float32r (same bits as f32; PE streams
    # f32r at 1 cycle/row for moving dims >= 256 vs 4 cycles/row for f32).
    W_d = nc.declare_dram_parameter("W", [2 * F, K], F32R, isOutput=False)
    b_d = nc.declare_dram_parameter("b", [1, K], F32R, isOutput=False)
    out_d = nc.declare_dram_parameter("out", [N, N * C * K], F32, isOutput=True)

    with tile.TileContext(nc) as tc, ExitStack() as ctx:
        const = ctx.enter_context(tc.tile_pool(name="const", bufs=1))
        psum_setup = ctx.enter_context(tc.tile_pool(name="psum_setup", bufs=1, space="PSUM"))
        psum_pair = ctx.enter_context(tc.tile_pool(name="psum_pair", bufs=3, space="PSUM"))
        sb_pair = ctx.enter_context(tc.tile_pool(name="sb_pair", bufs=3))
        sb_out = ctx.enter_context(tc.tile_pool(name="sb_out", bufs=3))

        h_t = const.tile([N, F], F32)
        nc.sync.dma_start(h_t[:], h_d[:])
        dist_t = const.tile([N, N * C], F32)
        nc.sync.dma_start(dist_t[:], dist_d[:])
        # W1 replicated 4x along free -> one [F, 512] f32r matmul covers 4 j's
        W1x4_t = const.tile([F, 4 * K], F32R)
        for q in range(4):
            nc.sync.dma_start(W1x4_t[:, q * K:(q + 1) * K], W_d[0:F, :])
        W2_t = const.tile([F, K], F32R)
        nc.sync.dma_start(W2_t[:], W_d[F:2 * F, :])
        b_t = const.tile([1, K], F32R)
        nc.sync.dma_start(b_t[:], b_d[:])

        ones_f32 = const.tile([1, N], F32)
        nc.vector.memset(ones_f32[:], 1.0)
        ones_t = const.tile([1, N], F32R)
        nc.gpsimd.dma_start(ones_t[:], ones_f32[:])
        ident = const.tile([N, N], F32)
        make_identity(nc, ident[:])

        # hT = h.T  (PE transpose); cast-DMA (gpsimd, bitwise) into f32r
        hT_ps = psum_setup.tile([F, N], F32, tag="hT_ps")
        nc.tensor.transpose(hT_ps[:], h_t[:], ident[:])
        hT_f32 = const.tile([F, N], F32)
        nc.scalar.copy(hT_f32[:], hT_ps[:])
        hT_t = const.tile([F, N], F32R)
        nc.gpsimd.dma_start(hT_t[:], hT_f32[:])

        # s2b = h @ W2 + bias (broadcast along partitions via ones x b)
        s2b_ps = psum_setup.tile([N, K], F32, tag="s2b_ps")
        nc.tensor.matmul(s2b_ps[:], hT_t[:], W2_t[:], start=True, stop=False)
        nc.tensor.matmul(s2b_ps[:], ones_t[:], b_t[:], start=False, stop=True)
        s2b_t = const.tile([N, K], F32)
        nc.scalar.copy(s2b_t[:], s2b_ps[:])
        # Flatten s2b rows onto partition 0 so row j is a free-dim slice
        # (compute-engine APs may only base at partition 0/32/64).
        s2b_flat = const.tile([1, N * K], F32R)
        nc.gpsimd.dma_start(s2b_flat[:], s2b_t[:])

        for g in range(NG * reps):
            g = g % NG
            pair_sb = sb_pair.tile([N, JG * K], F32, tag="pair")
            for half in range(JG // 4):
                pp = psum_pair.tile([N, 4 * K], F32, tag="pp")
                j0 = g * JG + half * 4
                nc.tensor.matmul(pp[:], hT_t[:], W1x4_t[:], start=True, stop=False)
                nc.tensor.matmul(pp[:], ones_t[:],
                                 s2b_flat[0:1, j0 * K:(j0 + 4) * K],
                                 start=False, stop=True)
                nc.scalar.copy(pair_sb[:, half * 4 * K:(half + 1) * 4 * K], pp[:])

            out_sb = sb_out.tile([N, JG * C * K], F32, tag="out")
            for dj in range(JG):
                j = g * JG + dj
                for c in range(C):
                    nc.vector.tensor_scalar_mul(
                        out_sb[:, (dj * C + c) * K:(dj * C + c + 1) * K],
                        pair_sb[:, dj * K:(dj + 1) * K],
                        dist_t[:, C * j + c:C * j + c + 1],
                    )
            nc.sync.dma_start(
                out_d[:, g * JG * C * K:(g + 1) * JG * C * K], out_sb[:])
    nc.finalize()
    return nc


def _run(scalar_features, distances, W, b, trace=False, reps=1):
    if ("nc", reps) not in _CACHE:
        _CACHE[("nc", reps)] = _build_nc(reps)
    nc = _CACHE[("nc", reps)]
    in_maps = []
    for i in range(B):
        in_maps.append({
            "h": np.ascontiguousarray(scalar_features[i], dtype=np.float32),
            "dist": np.ascontiguousarray(
                distances[i].reshape(N, N * C), dtype=np.float32),
            "W": np.ascontiguousarray(W, dtype=np.float32),
            "b": np.ascontiguousarray(np.asarray(b).reshape(1, K), dtype=np.float32),
        })
    r = run_bass_kernel_spmd(nc, in_maps, list(range(B)), trace=trace)
    out = np.stack([r.results[i]["out"].reshape(N, N, C, K) for i in range(B)])
    return out, r


def kernel(scalar_features, distances, W, b):
    out, _ = _run(scalar_features, distances, W, b, trace=False)
    return out
